# revision 7
# baseline (speedup 1.0000x reference)
"""Trainium2 Bass kernel for e3nn-style BatchNorm (instance norm over graphs).

Problem: x [200000, 480] f32, irreps 128x0e + 64x1o + 32x2e, batch_id sorted
into 64 graphs, weight [224], bias [128].

Math (per graph g):
  scalar block (cols 0:128):  m = mean_g(x); var = mean_g(x^2) - m^2
    A = w/sqrt(var+eps); B = b - m*A; out = x*A + B
  vector blocks (64x d=3, 32x d=5):  fn = mean_g(sum_d x^2)/d
    A = w/sqrt(fn+eps); out = x*A

Design: TRANSPOSED layout -- channels on partitions, rows along the free
dim, one padded slot per graph (8 slots/core, slot sizes = max row count
across cores so one SPMD program serves all 8 cores).  Per-graph stats are
then free-dim reductions (no one-hot matmuls), and the apply is a fused
per-partition tensor_scalar (no per-row param gather).  The PE and GpSimd
are idle; ACT does square+accumulate, DVE does sums and applies.

dtypes: scalar block ships f32 (mean subtraction needs ~2e-5 absolute
accuracy vs the 1e-3 denom clamp); vector blocks ship fp16 (pure scale,
error stays relative ~2^-11); ALL outputs fp16.  HBM traffic per core:
13.1 MB f32 in + 18 MB fp16 in + 24.7 MB fp16 out = 56 MB (vs 98.6 MB for
an all-f32 row-major kernel) -> ~156 us roofline at 358 GB/s.

Layouts (per core):
  x0 [128, N0] f32 : partition c = scalar channel; slot j at cols
     o0[j] : o0[j]+S[j] (rows of graph j, zero-padded to S[j]).
  x1 [128, N1] f16 : partition 64h+u = l=1 channel u, half h; slot j at
     o1[j] : o1[j]+3*H[j], d-innermost ([row][d]); slot rows split at H[j].
  x2 [128, N2] f16 : partition 32q+u = l=2 channel u, quarter q; 5*Q[j].
  outputs o0/o1/o2: same layouts, fp16.
Per-slot sums for halves/quarters are combined (and broadcast back to all
128 partitions) with one tiny PE matmul against a mod-64 / mod-32 0-1
matrix.  Counts ship from host (same batch_id bookkeeping as the
searchsorted sharding itself).
"""

import sys

if "/opt/trn_rl_repo" not in sys.path:
    sys.path.insert(0, "/opt/trn_rl_repo")

import numpy as np

P = 128
NCORES = 8
G = 64
NSLOT = G // NCORES  # graph slots per core
EPS = 1e-5
SLOT_BUFS = 4

_prog_cache = {}


def _build(S):
    """S: tuple of NSLOT slot sizes (each a multiple of 8)."""
    import concourse.bacc as bacc
    import concourse.tile as tile
    from concourse import mybir

    f32 = mybir.dt.float32
    f16 = mybir.dt.float16
    Alu = mybir.AluOpType
    Act = mybir.ActivationFunctionType
    X = mybir.AxisListType.X

    H = [s // 2 for s in S]
    Q = [s // 4 for s in S]
    o0 = np.concatenate([[0], np.cumsum(S)]).astype(int)
    o1 = np.concatenate([[0], np.cumsum([3 * h for h in H])]).astype(int)
    o2 = np.concatenate([[0], np.cumsum([5 * q for q in Q])]).astype(int)
    N0, N1, N2 = int(o0[-1]), int(o1[-1]), int(o2[-1])
    Smax, Hmax, Qmax = max(S), max(H), max(Q)

    nc = bacc.Bacc("TRN2", target_bir_lowering=False, debug=False,
                   num_devices=NCORES)
    t_x0 = nc.dram_tensor("x0", [P, N0], f32, kind="ExternalInput")
    t_x1 = nc.dram_tensor("x1", [P, N1], f16, kind="ExternalInput")
    t_x2 = nc.dram_tensor("x2", [P, N2], f16, kind="ExternalInput")
    t_wsb = nc.dram_tensor("wsb", [P, 4], f32, kind="ExternalInput")
    t_cnt = nc.dram_tensor("cnt", [P, NSLOT], f32, kind="ExternalInput")
    t_mm = nc.dram_tensor("mm", [P, 256], f32, kind="ExternalInput")
    t_y0 = nc.dram_tensor("y0", [P, N0], f16, kind="ExternalOutput")
    t_y1 = nc.dram_tensor("y1", [P, N1], f16, kind="ExternalOutput")
    t_y2 = nc.dram_tensor("y2", [P, N2], f16, kind="ExternalOutput")

    with tile.TileContext(nc) as tc:
        with (
            tc.tile_pool(name="const", bufs=1) as cp,
            tc.tile_pool(name="slot", bufs=SLOT_BUFS) as sp,
            tc.tile_pool(name="scr", bufs=1) as scp,
            tc.tile_pool(name="st", bufs=SLOT_BUFS) as stp,
            tc.tile_pool(name="par", bufs=2) as pp,
            tc.tile_pool(name="ps", bufs=2, space="PSUM") as ps,
        ):
            wsb = cp.tile([P, 4], f32, tag="wsb")
            nc.sync.dma_start(out=wsb[:], in_=t_wsb.ap())
            cnt = cp.tile([P, NSLOT], f32, tag="cnt")
            nc.sync.dma_start(out=cnt[:], in_=t_cnt.ap())
            mmt = cp.tile([P, 256], f32, tag="mmt")
            nc.sync.dma_start(out=mmt[:], in_=t_mm.ap())
            eps_t = cp.tile([P, 1], f32, tag="eps")
            nc.vector.memset(eps_t[:], EPS)
            invn = cp.tile([P, NSLOT], f32, tag="invn")
            nc.vector.tensor_scalar_max(out=invn[:], in0=cnt[:], scalar1=1.0)
            nc.vector.reciprocal(out=invn[:], in_=invn[:])

            slots = {}

            def load_and_stats(j):
                x0t = sp.tile([P, Smax], f32, tag="x0t")
                x1t = sp.tile([P, 3 * Hmax], f16, tag="x1t")
                x2t = sp.tile([P, 5 * Qmax], f16, tag="x2t")
                yst = sp.tile([P, Smax], f16, tag="yst")
                nc.sync.dma_start(out=x0t[:, 0:S[j]],
                                  in_=t_x0.ap()[:, o0[j]:o0[j] + S[j]])
                nc.sync.dma_start(out=x1t[:, 0:3 * H[j]],
                                  in_=t_x1.ap()[:, o1[j]:o1[j] + 3 * H[j]])
                nc.sync.dma_start(out=x2t[:, 0:5 * Q[j]],
                                  in_=t_x2.ap()[:, o2[j]:o2[j] + 5 * Q[j]])
                # stats in two half-chunks so the last chunk's squares are
                # short; stp: sum_x | sum_x2 | sum_v1 | sum_v2 (x2 halves)
                st = stp.tile([P, 8], f32, tag="st")
                c0 = (S[j] // 16) * 8
                c1 = 3 * ((H[j] // 2))
                c2 = 5 * ((Q[j] // 2))
                scr0 = scp.tile([P, Smax], f16, tag="scr0")
                scr1 = scp.tile([P, 3 * Hmax], f16, tag="scr1")
                scr2 = scp.tile([P, 5 * Qmax], f16, tag="scr2")
                for h, (a0, b0, a1, b1, a2, b2) in enumerate(
                        ((0, c0, 0, c1, 0, c2),
                         (c0, S[j], c1, 3 * H[j], c2, 5 * Q[j]))):
                    o = 4 * h
                    nc.scalar.activation(out=scr0[:, a0:b0],
                                         in_=x0t[:, a0:b0], func=Act.Square,
                                         accum_out=st[:, o + 1:o + 2])
                    nc.scalar.activation(out=scr1[:, a1:b1],
                                         in_=x1t[:, a1:b1], func=Act.Square,
                                         accum_out=st[:, o + 2:o + 3])
                    nc.scalar.activation(out=scr2[:, a2:b2],
                                         in_=x2t[:, a2:b2], func=Act.Square,
                                         accum_out=st[:, o + 3:o + 4])
                    nc.vector.tensor_reduce(out=st[:, o:o + 1],
                                            in_=x0t[:, a0:b0],
                                            axis=X, op=Alu.add)
                nc.vector.tensor_tensor(out=st[:, 0:4], in0=st[:, 0:4],
                                        in1=st[:, 4:8], op=Alu.add)
                # combine halves/quarters across partitions (and broadcast)
                cmb = ps.tile([P, 2], f32, tag="cmb")
                nc.tensor.matmul(out=cmb[:, 0:1], lhsT=mmt[:, 0:128],
                                 rhs=st[:, 2:3], start=True, stop=True)
                nc.tensor.matmul(out=cmb[:, 1:2], lhsT=mmt[:, 128:256],
                                 rhs=st[:, 3:4], start=True, stop=True)
                slots[j] = (x0t, x1t, x2t, yst, st, cmb)

            def params_apply_store(j):
                x0t, x1t, x2t, yst, st, cmb = slots.pop(j)
                ivj = invn[:, j:j + 1]
                t = lambda name: pp.tile([P, 1], f32, tag=name, name=name)
                # scalar block: m, var -> A_s, B_s
                me = pp.tile([P, 2], f32, tag="me")
                nc.vector.tensor_scalar_mul(out=me[:], in0=st[:, 0:2],
                                            scalar1=ivj)
                m2 = t("m2")
                nc.vector.tensor_tensor(out=m2[:], in0=me[:, 0:1],
                                        in1=me[:, 0:1], op=Alu.mult)
                var = t("var")
                nc.vector.tensor_tensor(out=var[:], in0=me[:, 1:2],
                                        in1=m2[:], op=Alu.subtract)
                nc.vector.tensor_scalar_max(out=var[:], in0=var[:],
                                            scalar1=0.0)
                nc.scalar.activation(out=var[:], in_=var[:], func=Act.Sqrt,
                                     bias=eps_t[:])
                nc.vector.reciprocal(out=var[:], in_=var[:])
                A_s = t("A_s")
                nc.vector.tensor_tensor(out=A_s[:], in0=var[:],
                                        in1=wsb[:, 0:1], op=Alu.mult)
                mA = t("mA")
                nc.vector.tensor_tensor(out=mA[:], in0=me[:, 0:1],
                                        in1=A_s[:], op=Alu.mult)
                B_s = t("B_s")
                nc.vector.tensor_tensor(out=B_s[:], in0=wsb[:, 1:2],
                                        in1=mA[:], op=Alu.subtract)
                # vector blocks: fn -> A_1, A_2
                fv = pp.tile([P, 2], f32, tag="fv")
                nc.vector.tensor_scalar(out=fv[:, 0:1], in0=cmb[:, 0:1],
                                        scalar1=ivj, scalar2=1.0 / 3.0,
                                        op0=Alu.mult, op1=Alu.mult)
                nc.vector.tensor_scalar(out=fv[:, 1:2], in0=cmb[:, 1:2],
                                        scalar1=ivj, scalar2=1.0 / 5.0,
                                        op0=Alu.mult, op1=Alu.mult)
                nc.scalar.activation(out=fv[:], in_=fv[:], func=Act.Sqrt,
                                     bias=eps_t[:])
                nc.vector.reciprocal(out=fv[:], in_=fv[:])
                A_v = pp.tile([P, 2], f32, tag="A_v")
                nc.vector.tensor_tensor(out=A_v[:], in0=fv[:],
                                        in1=wsb[:, 2:4], op=Alu.mult)
                # apply + store, in two half-chunks for finer DMA overlap
                c0 = (S[j] // 16) * 8
                c1 = 3 * ((H[j] // 2))
                c2 = 5 * ((Q[j] // 2))
                for a0, b0, a1, b1, a2, b2 in (
                        (0, c0, 0, c1, 0, c2),
                        (c0, S[j], c1, 3 * H[j], c2, 5 * Q[j])):
                    nc.vector.tensor_scalar(out=yst[:, a0:b0],
                                            in0=x0t[:, a0:b0],
                                            scalar1=A_s[:], scalar2=B_s[:],
                                            op0=Alu.mult, op1=Alu.add)
                    nc.scalar.dma_start(
                        out=t_y0.ap()[:, o0[j] + a0:o0[j] + b0],
                        in_=yst[:, a0:b0])
                    nc.vector.tensor_scalar_mul(out=x1t[:, a1:b1],
                                                in0=x1t[:, a1:b1],
                                                scalar1=A_v[:, 0:1])
                    nc.scalar.dma_start(
                        out=t_y1.ap()[:, o1[j] + a1:o1[j] + b1],
                        in_=x1t[:, a1:b1])
                    nc.vector.tensor_scalar_mul(out=x2t[:, a2:b2],
                                                in0=x2t[:, a2:b2],
                                                scalar1=A_v[:, 1:2])
                    nc.scalar.dma_start(
                        out=t_y2.ap()[:, o2[j] + a2:o2[j] + b2],
                        in_=x2t[:, a2:b2])

            # software-pipelined: params/apply for slot j-1 issue after
            # stats for slot j, so no engine queue stalls on short deps
            for j in range(NSLOT):
                load_and_stats(j)
                if j >= 1:
                    params_apply_store(j - 1)
            params_apply_store(NSLOT - 1)

    nc.compile()
    return nc, (o0, o1, o2)


def kernel(input, batch_id_tensor, weight, bias, _trace=False):
    from concourse import bass_utils

    x = np.asarray(input, dtype=np.float32)
    bid = np.asarray(batch_id_tensor).astype(np.int64)
    w = np.asarray(weight, dtype=np.float32)
    b = np.asarray(bias, dtype=np.float32)
    n = x.shape[0]

    edges = np.searchsorted(bid, np.arange(G + 1), side="left")
    counts = np.diff(edges)  # per-graph row counts
    # slot j size = max over cores of count(graph c*NSLOT+j), mult of 8
    cnt_mat = counts.reshape(NCORES, NSLOT)
    S = tuple(int(max(8, -(-int(cnt_mat[:, j].max()) // 8) * 8))
              for j in range(NSLOT))

    if S not in _prog_cache:
        _prog_cache[S] = _build(S)
    nc, (o0, o1, o2) = _prog_cache[S]

    H = [s // 2 for s in S]
    Q = [s // 4 for s in S]
    N0, N1, N2 = int(o0[-1]), int(o1[-1]), int(o2[-1])

    wsb = np.zeros((P, 4), np.float32)
    wsb[:, 0] = w[0:128]
    wsb[:, 1] = b
    wsb[:, 2] = np.tile(w[128:192], 2)
    wsb[:, 3] = np.tile(w[192:224], 4)
    mm = np.zeros((P, 256), np.float32)
    k = np.arange(P)
    mm[:, 0:128] = (k[:, None] % 64 == k[None, :] % 64)
    mm[:, 128:256] = (k[:, None] % 32 == k[None, :] % 32)

    in_maps = []
    for c in range(NCORES):
        xa0 = np.zeros((P, N0), np.float32)
        xa1 = np.zeros((P, N1), np.float16)
        xa2 = np.zeros((P, N2), np.float16)
        for j in range(NSLOT):
            g = c * NSLOT + j
            lo, hi = int(edges[g]), int(edges[g + 1])
            ncj = hi - lo
            if ncj == 0:
                continue
            xa0[:, o0[j]:o0[j] + ncj] = x[lo:hi, 0:128].T
            b1 = np.ascontiguousarray(x[lo:hi, 128:320]).astype(np.float16)
            b1 = b1.reshape(ncj, 64, 3)
            r0 = min(H[j], ncj)
            xa1[0:64, o1[j]:o1[j] + 3 * r0] = \
                b1[:r0].transpose(1, 0, 2).reshape(64, 3 * r0)
            if ncj > H[j]:
                r1 = ncj - H[j]
                xa1[64:128, o1[j]:o1[j] + 3 * r1] = \
                    b1[H[j]:].transpose(1, 0, 2).reshape(64, 3 * r1)
            b2 = np.ascontiguousarray(x[lo:hi, 320:480]).astype(np.float16)
            b2 = b2.reshape(ncj, 32, 5)
            for q in range(4):
                qlo, qhi = q * Q[j], min((q + 1) * Q[j], ncj)
                if qhi <= qlo:
                    break
                xa2[32 * q:32 * (q + 1), o2[j]:o2[j] + 5 * (qhi - qlo)] = \
                    b2[qlo:qhi].transpose(1, 0, 2).reshape(32, 5 * (qhi - qlo))
        cnt = np.broadcast_to(
            cnt_mat[c].astype(np.float32), (P, NSLOT)).copy()
        in_maps.append({"x0": xa0, "x1": xa1, "x2": xa2,
                        "wsb": wsb, "cnt": cnt, "mm": mm})

    res = bass_utils.run_bass_kernel_spmd(
        nc, in_maps, core_ids=list(range(NCORES)), trace=_trace)

    out = np.empty((n, 480), np.float32)
    for c in range(NCORES):
        r = res.results[c]
        y0, y1, y2 = r["y0"], r["y1"], r["y2"]
        for j in range(NSLOT):
            g = c * NSLOT + j
            lo, hi = int(edges[g]), int(edges[g + 1])
            ncj = hi - lo
            if ncj == 0:
                continue
            out[lo:hi, 0:128] = y0[:, o0[j]:o0[j] + ncj].T
            r0 = min(H[j], ncj)
            out[lo:lo + r0, 128:320] = \
                y1[0:64, o1[j]:o1[j] + 3 * r0].reshape(
                    64, r0, 3).transpose(1, 0, 2).reshape(r0, 192)
            if ncj > H[j]:
                r1 = ncj - H[j]
                out[lo + H[j]:hi, 128:320] = \
                    y1[64:128, o1[j]:o1[j] + 3 * r1].reshape(
                        64, r1, 3).transpose(1, 0, 2).reshape(r1, 192)
            for q in range(4):
                qlo, qhi = q * Q[j], min((q + 1) * Q[j], ncj)
                if qhi <= qlo:
                    break
                rq = qhi - qlo
                out[lo + qlo:lo + qhi, 320:480] = \
                    y2[32 * q:32 * (q + 1), o2[j]:o2[j] + 5 * rq].reshape(
                        32, rq, 5).transpose(1, 0, 2).reshape(rq, 160)
    if _trace:
        return out, res
    return out


# revision 9
# speedup vs baseline: 1.3084x; 1.3084x over previous
"""Trainium2 Bass kernel for e3nn-style BatchNorm (instance norm over graphs).

Problem: x [200000, 480] f32, irreps 128x0e + 64x1o + 32x2e, batch_id sorted
into 64 graphs, weight [224], bias [128].

Math (per graph g):
  scalar block (cols 0:128):  m = mean_g(x); var = mean_g(x^2) - m^2
    A = w/sqrt(var+eps); B = b - m*A; out = x*A + B
  vector blocks (64x d=3, 32x d=5):  fn = mean_g(sum_d x^2)/d
    A = w/sqrt(fn+eps); out = x*A

Design: TRANSPOSED layout -- channels on partitions, rows along the free
dim, one padded slot per graph (8 slots/core, slot sizes = max row count
across cores so one SPMD program serves all 8 cores).  Per-graph stats are
then free-dim reductions (no one-hot matmuls), and the apply is a fused
per-partition tensor_scalar (no per-row param gather).  The PE and GpSimd
are idle; ACT does square+accumulate, DVE does sums and applies.

dtypes: scalar block ships f32 (mean subtraction needs ~2e-5 absolute
accuracy vs the 1e-3 denom clamp); vector blocks ship fp16 (pure scale,
error stays relative ~2^-11); ALL outputs fp16.  HBM traffic per core:
13.1 MB f32 in + 18 MB fp16 in + 24.7 MB fp16 out = 56 MB (vs 98.6 MB for
an all-f32 row-major kernel) -> ~156 us roofline at 358 GB/s.

Layouts (per core):
  x0 [128, N0] f32 : partition c = scalar channel; slot j at cols
     o0[j] : o0[j]+S[j] (rows of graph j, zero-padded to S[j]).
  x1 [128, N1] f16 : partition 64h+u = l=1 channel u, half h; slot j at
     o1[j] : o1[j]+3*H[j], d-innermost ([row][d]); slot rows split at H[j].
  x2 [128, N2] f16 : partition 32q+u = l=2 channel u, quarter q; 5*Q[j].
  outputs o0/o1/o2: same layouts, fp16.
Per-slot sums for halves/quarters are combined (and broadcast back to all
128 partitions) with one tiny PE matmul against a mod-64 / mod-32 0-1
matrix.  Counts ship from host (same batch_id bookkeeping as the
searchsorted sharding itself).
"""

import sys

if "/opt/trn_rl_repo" not in sys.path:
    sys.path.insert(0, "/opt/trn_rl_repo")

import numpy as np

P = 128
NCORES = 8
G = 64
NSLOT = G // NCORES  # graph slots per core
EPS = 1e-5
SLOT_BUFS = 4

_prog_cache = {}


def _build(S):
    """S: tuple of NSLOT slot sizes (each a multiple of 8)."""
    import concourse.bacc as bacc
    import concourse.tile as tile
    from concourse import mybir

    f32 = mybir.dt.float32
    f16 = mybir.dt.float16
    Alu = mybir.AluOpType
    Act = mybir.ActivationFunctionType
    X = mybir.AxisListType.X

    H = [s // 2 for s in S]
    Q = [s // 4 for s in S]
    o0 = np.concatenate([[0], np.cumsum(S)]).astype(int)
    o1 = np.concatenate([[0], np.cumsum([3 * h for h in H])]).astype(int)
    o2 = np.concatenate([[0], np.cumsum([5 * q for q in Q])]).astype(int)
    N0, N1, N2 = int(o0[-1]), int(o1[-1]), int(o2[-1])
    Smax, Hmax, Qmax = max(S), max(H), max(Q)

    nc = bacc.Bacc("TRN2", target_bir_lowering=False, debug=False,
                   num_devices=NCORES)
    t_x0 = nc.dram_tensor("x0", [P, N0], f32, kind="ExternalInput")
    t_x1 = nc.dram_tensor("x1", [P, N1], f16, kind="ExternalInput")
    t_x2 = nc.dram_tensor("x2", [P, N2], f16, kind="ExternalInput")
    t_wsb = nc.dram_tensor("wsb", [P, 4], f32, kind="ExternalInput")
    t_cnt = nc.dram_tensor("cnt", [P, NSLOT], f32, kind="ExternalInput")
    t_mm = nc.dram_tensor("mm", [P, 256], f32, kind="ExternalInput")
    t_y0 = nc.dram_tensor("y0", [P, N0], f16, kind="ExternalOutput")
    t_y1 = nc.dram_tensor("y1", [P, N1], f16, kind="ExternalOutput")
    t_y2 = nc.dram_tensor("y2", [P, N2], f16, kind="ExternalOutput")

    with tile.TileContext(nc) as tc:
        with (
            tc.tile_pool(name="const", bufs=1) as cp,
            tc.tile_pool(name="slot", bufs=SLOT_BUFS) as sp,
            tc.tile_pool(name="scr", bufs=1) as scp,
            tc.tile_pool(name="st", bufs=SLOT_BUFS) as stp,
            tc.tile_pool(name="par", bufs=2) as pp,
            tc.tile_pool(name="ps", bufs=2, space="PSUM") as ps,
        ):
            wsb = cp.tile([P, 4], f32, tag="wsb")
            nc.sync.dma_start(out=wsb[:], in_=t_wsb.ap())
            cnt = cp.tile([P, NSLOT], f32, tag="cnt")
            nc.sync.dma_start(out=cnt[:], in_=t_cnt.ap())
            mmt = cp.tile([P, 256], f32, tag="mmt")
            nc.sync.dma_start(out=mmt[:], in_=t_mm.ap())
            eps_t = cp.tile([P, 1], f32, tag="eps")
            nc.vector.memset(eps_t[:], EPS)
            invn = cp.tile([P, NSLOT], f32, tag="invn")
            nc.vector.tensor_scalar_max(out=invn[:], in0=cnt[:], scalar1=1.0)
            nc.vector.reciprocal(out=invn[:], in_=invn[:])

            slots = {}

            def load_and_stats(j):
                x0t = sp.tile([P, Smax], f32, tag="x0t")
                x1t = sp.tile([P, 3 * Hmax], f16, tag="x1t")
                x2t = sp.tile([P, 5 * Qmax], f16, tag="x2t")
                yst = sp.tile([P, Smax], f16, tag="yst")
                nc.sync.dma_start(out=x0t[:, 0:S[j]],
                                  in_=t_x0.ap()[:, o0[j]:o0[j] + S[j]])
                nc.sync.dma_start(out=x1t[:, 0:3 * H[j]],
                                  in_=t_x1.ap()[:, o1[j]:o1[j] + 3 * H[j]])
                nc.sync.dma_start(out=x2t[:, 0:5 * Q[j]],
                                  in_=t_x2.ap()[:, o2[j]:o2[j] + 5 * Q[j]])
                # stats in two half-chunks so the last chunk's squares are
                # short; stp: sum_x | sum_x2 | sum_v1 | sum_v2 (x2 halves)
                st = stp.tile([P, 8], f32, tag="st")
                c0 = (S[j] // 16) * 8
                c1 = 3 * ((H[j] // 2))
                c2 = 5 * ((Q[j] // 2))
                scr0 = scp.tile([P, Smax], f16, tag="scr0")
                scr1 = scp.tile([P, 3 * Hmax], f16, tag="scr1")
                scr2 = scp.tile([P, 5 * Qmax], f16, tag="scr2")
                for h, (a0, b0, a1, b1, a2, b2) in enumerate(
                        ((0, c0, 0, c1, 0, c2),
                         (c0, S[j], c1, 3 * H[j], c2, 5 * Q[j]))):
                    o = 4 * h
                    nc.scalar.activation(out=scr0[:, a0:b0],
                                         in_=x0t[:, a0:b0], func=Act.Square,
                                         accum_out=st[:, o + 1:o + 2])
                    nc.scalar.activation(out=scr1[:, a1:b1],
                                         in_=x1t[:, a1:b1], func=Act.Square,
                                         accum_out=st[:, o + 2:o + 3])
                    nc.scalar.activation(out=scr2[:, a2:b2],
                                         in_=x2t[:, a2:b2], func=Act.Square,
                                         accum_out=st[:, o + 3:o + 4])
                    nc.vector.tensor_reduce(out=st[:, o:o + 1],
                                            in_=x0t[:, a0:b0],
                                            axis=X, op=Alu.add)
                nc.vector.tensor_tensor(out=st[:, 0:4], in0=st[:, 0:4],
                                        in1=st[:, 4:8], op=Alu.add)
                # combine halves/quarters across partitions (and broadcast)
                cmb = ps.tile([P, 2], f32, tag="cmb")
                nc.tensor.matmul(out=cmb[:, 0:1], lhsT=mmt[:, 0:128],
                                 rhs=st[:, 2:3], start=True, stop=True)
                nc.tensor.matmul(out=cmb[:, 1:2], lhsT=mmt[:, 128:256],
                                 rhs=st[:, 3:4], start=True, stop=True)
                slots[j] = (x0t, x1t, x2t, yst, st, cmb)

            def params_apply_store(j):
                x0t, x1t, x2t, yst, st, cmb = slots.pop(j)
                ivj = invn[:, j:j + 1]
                t = lambda name: pp.tile([P, 1], f32, tag=name, name=name)
                # scalar block: m, var -> A_s, B_s
                me = pp.tile([P, 2], f32, tag="me")
                nc.vector.tensor_scalar_mul(out=me[:], in0=st[:, 0:2],
                                            scalar1=ivj)
                m2 = t("m2")
                nc.vector.tensor_tensor(out=m2[:], in0=me[:, 0:1],
                                        in1=me[:, 0:1], op=Alu.mult)
                var = t("var")
                nc.vector.tensor_tensor(out=var[:], in0=me[:, 1:2],
                                        in1=m2[:], op=Alu.subtract)
                nc.vector.tensor_scalar_max(out=var[:], in0=var[:],
                                            scalar1=0.0)
                nc.scalar.activation(out=var[:], in_=var[:], func=Act.Sqrt,
                                     bias=eps_t[:])
                nc.vector.reciprocal(out=var[:], in_=var[:])
                A_s = t("A_s")
                nc.vector.tensor_tensor(out=A_s[:], in0=var[:],
                                        in1=wsb[:, 0:1], op=Alu.mult)
                mA = t("mA")
                nc.vector.tensor_tensor(out=mA[:], in0=me[:, 0:1],
                                        in1=A_s[:], op=Alu.mult)
                B_s = t("B_s")
                nc.vector.tensor_tensor(out=B_s[:], in0=wsb[:, 1:2],
                                        in1=mA[:], op=Alu.subtract)
                # vector blocks: fn -> A_1, A_2
                fv = pp.tile([P, 2], f32, tag="fv")
                nc.vector.tensor_scalar(out=fv[:, 0:1], in0=cmb[:, 0:1],
                                        scalar1=ivj, scalar2=1.0 / 3.0,
                                        op0=Alu.mult, op1=Alu.mult)
                nc.vector.tensor_scalar(out=fv[:, 1:2], in0=cmb[:, 1:2],
                                        scalar1=ivj, scalar2=1.0 / 5.0,
                                        op0=Alu.mult, op1=Alu.mult)
                nc.scalar.activation(out=fv[:], in_=fv[:], func=Act.Sqrt,
                                     bias=eps_t[:])
                nc.vector.reciprocal(out=fv[:], in_=fv[:])
                A_v = pp.tile([P, 2], f32, tag="A_v")
                nc.vector.tensor_tensor(out=A_v[:], in0=fv[:],
                                        in1=wsb[:, 2:4], op=Alu.mult)
                # apply + store, in two half-chunks for finer DMA overlap
                c0 = (S[j] // 16) * 8
                c1 = 3 * ((H[j] // 2))
                c2 = 5 * ((Q[j] // 2))
                for a0, b0, a1, b1, a2, b2 in (
                        (0, c0, 0, c1, 0, c2),
                        (c0, S[j], c1, 3 * H[j], c2, 5 * Q[j])):
                    nc.vector.tensor_scalar(out=yst[:, a0:b0],
                                            in0=x0t[:, a0:b0],
                                            scalar1=A_s[:], scalar2=B_s[:],
                                            op0=Alu.mult, op1=Alu.add)
                    nc.gpsimd.dma_start(
                        out=t_y0.ap()[:, o0[j] + a0:o0[j] + b0],
                        in_=yst[:, a0:b0])
                    nc.vector.tensor_scalar_mul(out=x1t[:, a1:b1],
                                                in0=x1t[:, a1:b1],
                                                scalar1=A_v[:, 0:1])
                    nc.gpsimd.dma_start(
                        out=t_y1.ap()[:, o1[j] + a1:o1[j] + b1],
                        in_=x1t[:, a1:b1])
                    nc.vector.tensor_scalar_mul(out=x2t[:, a2:b2],
                                                in0=x2t[:, a2:b2],
                                                scalar1=A_v[:, 1:2])
                    nc.gpsimd.dma_start(
                        out=t_y2.ap()[:, o2[j] + a2:o2[j] + b2],
                        in_=x2t[:, a2:b2])

            # per slot: loads -> stats -> params -> apply -> store.  Each
            # engine queue only waits on work from its own or earlier
            # slots (stores sit on the idle GpSimd/SWDGE queue), so the
            # cross-slot cascade stays off the DMA critical path.
            for j in range(NSLOT):
                load_and_stats(j)
                params_apply_store(j)

    nc.compile()
    return nc, (o0, o1, o2)


def kernel(input, batch_id_tensor, weight, bias, _trace=False):
    from concourse import bass_utils

    x = np.asarray(input, dtype=np.float32)
    bid = np.asarray(batch_id_tensor).astype(np.int64)
    w = np.asarray(weight, dtype=np.float32)
    b = np.asarray(bias, dtype=np.float32)
    n = x.shape[0]

    edges = np.searchsorted(bid, np.arange(G + 1), side="left")
    counts = np.diff(edges)  # per-graph row counts
    # slot j size = max over cores of count(graph c*NSLOT+j), mult of 8
    cnt_mat = counts.reshape(NCORES, NSLOT)
    S = tuple(int(max(8, -(-int(cnt_mat[:, j].max()) // 8) * 8))
              for j in range(NSLOT))

    if S not in _prog_cache:
        _prog_cache[S] = _build(S)
    nc, (o0, o1, o2) = _prog_cache[S]

    H = [s // 2 for s in S]
    Q = [s // 4 for s in S]
    N0, N1, N2 = int(o0[-1]), int(o1[-1]), int(o2[-1])

    wsb = np.zeros((P, 4), np.float32)
    wsb[:, 0] = w[0:128]
    wsb[:, 1] = b
    wsb[:, 2] = np.tile(w[128:192], 2)
    wsb[:, 3] = np.tile(w[192:224], 4)
    mm = np.zeros((P, 256), np.float32)
    k = np.arange(P)
    mm[:, 0:128] = (k[:, None] % 64 == k[None, :] % 64)
    mm[:, 128:256] = (k[:, None] % 32 == k[None, :] % 32)

    in_maps = []
    for c in range(NCORES):
        xa0 = np.zeros((P, N0), np.float32)
        xa1 = np.zeros((P, N1), np.float16)
        xa2 = np.zeros((P, N2), np.float16)
        for j in range(NSLOT):
            g = c * NSLOT + j
            lo, hi = int(edges[g]), int(edges[g + 1])
            ncj = hi - lo
            if ncj == 0:
                continue
            xa0[:, o0[j]:o0[j] + ncj] = x[lo:hi, 0:128].T
            b1 = np.ascontiguousarray(x[lo:hi, 128:320]).astype(np.float16)
            b1 = b1.reshape(ncj, 64, 3)
            r0 = min(H[j], ncj)
            xa1[0:64, o1[j]:o1[j] + 3 * r0] = \
                b1[:r0].transpose(1, 0, 2).reshape(64, 3 * r0)
            if ncj > H[j]:
                r1 = ncj - H[j]
                xa1[64:128, o1[j]:o1[j] + 3 * r1] = \
                    b1[H[j]:].transpose(1, 0, 2).reshape(64, 3 * r1)
            b2 = np.ascontiguousarray(x[lo:hi, 320:480]).astype(np.float16)
            b2 = b2.reshape(ncj, 32, 5)
            for q in range(4):
                qlo, qhi = q * Q[j], min((q + 1) * Q[j], ncj)
                if qhi <= qlo:
                    break
                xa2[32 * q:32 * (q + 1), o2[j]:o2[j] + 5 * (qhi - qlo)] = \
                    b2[qlo:qhi].transpose(1, 0, 2).reshape(32, 5 * (qhi - qlo))
        cnt = np.broadcast_to(
            cnt_mat[c].astype(np.float32), (P, NSLOT)).copy()
        in_maps.append({"x0": xa0, "x1": xa1, "x2": xa2,
                        "wsb": wsb, "cnt": cnt, "mm": mm})

    res = bass_utils.run_bass_kernel_spmd(
        nc, in_maps, core_ids=list(range(NCORES)), trace=_trace)

    out = np.empty((n, 480), np.float32)
    for c in range(NCORES):
        r = res.results[c]
        y0, y1, y2 = r["y0"], r["y1"], r["y2"]
        for j in range(NSLOT):
            g = c * NSLOT + j
            lo, hi = int(edges[g]), int(edges[g + 1])
            ncj = hi - lo
            if ncj == 0:
                continue
            out[lo:hi, 0:128] = y0[:, o0[j]:o0[j] + ncj].T
            r0 = min(H[j], ncj)
            out[lo:lo + r0, 128:320] = \
                y1[0:64, o1[j]:o1[j] + 3 * r0].reshape(
                    64, r0, 3).transpose(1, 0, 2).reshape(r0, 192)
            if ncj > H[j]:
                r1 = ncj - H[j]
                out[lo + H[j]:hi, 128:320] = \
                    y1[64:128, o1[j]:o1[j] + 3 * r1].reshape(
                        64, r1, 3).transpose(1, 0, 2).reshape(r1, 192)
            for q in range(4):
                qlo, qhi = q * Q[j], min((q + 1) * Q[j], ncj)
                if qhi <= qlo:
                    break
                rq = qhi - qlo
                out[lo + qlo:lo + qhi, 320:480] = \
                    y2[32 * q:32 * (q + 1), o2[j]:o2[j] + 5 * rq].reshape(
                        32, rq, 5).transpose(1, 0, 2).reshape(rq, 160)
    if _trace:
        return out, res
    return out
